# revision 19
# baseline (speedup 1.0000x reference)
"""Blockwise transformer attention layer on 8 trn2 NeuronCores.

Math (per reference):
    q = (x @ Wq.T) / sqrt(D); k = x @ Wk.T; v = x @ Wv.T       (B,S,D), H=16 heads of Dh=64
    out = softmax(q k^T per head) @ v                           (no causal mask; scores ~ N(0,1/16)
                                                                 so exp without max-subtraction)
    y = out @ Wff.T + bff

Sharding: tensor-parallel over heads. 8 cores x 2 heads each. Each core:
  - computes qT,kT (transposed, [128=2*Dh, S]) and v (natural, [S,128]) for its 2 heads
    from the full xT and its weight slices,
  - attention with scores materialized TRANSPOSED ([k_pos, q_pos]) so exp(scores)
    feeds the o^T = v^T @ P accumulation directly (no PE transposes),
  - softmax denominator comes free from a ones-column appended to v,
  - partial final projection partial^T = Wff[:, slice].T-contraction, written transposed (bf16).
Host sums the 8 partials in fp32, transposes back, adds bias.

Schedule: emission interleaves the ACT(exp)-paced attention inner loops with the
QKV matmuls of the next batch / FF matmuls of the previous batch so the PE never
idles long enough for the HAM clock gate to re-throttle it.
"""

import numpy as np
import ml_dtypes

BF16 = ml_dtypes.bfloat16

B, S, D = 2, 2048, 1024
DH = 64          # head dim
HPC = 2          # heads per core
NCORES = 8
NQ = 512         # q-chunk width (psum bank width in fp32)


def _interleave(primary, filler, skip=0):
    """Emit primary units in order with filler units spread evenly between
    them, starting after the first `skip` primary units."""
    out = []
    np_, nf = len(primary), len(filler)
    span = max(np_ - skip, 1)
    fi = 0
    for i, u in enumerate(primary):
        out.append(u)
        want = max(i + 1 - skip, 0) * nf // span
        while fi < want:
            out.append(filler[fi])
            fi += 1
    out.extend(filler[fi:])
    return out


def build_program(b=B, s=S, d=D, num_devices=NCORES, debug=False):
    import concourse.bass as bass
    import concourse.tile as tile
    from concourse import bacc, mybir
    from concourse._compat import get_trn_type
    from contextlib import ExitStack

    f32 = mybir.dt.float32
    bf16 = mybir.dt.bfloat16
    Exp = mybir.ActivationFunctionType.Exp

    KC = d // 128           # contraction chunks over D
    SQ = s // NQ            # q chunks
    SB = s // 128           # k blocks
    VW = DH + 1             # v block cols per head (64 dims + ones col)
    NG = HPC * SQ           # normalization groups per batch

    nc = bacc.Bacc(
        get_trn_type() or "TRN2",
        target_bir_lowering=False,
        debug=debug,
        num_devices=num_devices,
    )

    xT = nc.dram_tensor("xT", (b, KC, 128, s), bf16, kind="ExternalInput")
    wqT = nc.dram_tensor("wqT", (KC, 128, 128), bf16, kind="ExternalInput")
    wkT = nc.dram_tensor("wkT", (KC, 128, 128), bf16, kind="ExternalInput")
    wvT = nc.dram_tensor("wvT", (KC, 128, 128), bf16, kind="ExternalInput")
    wfT = nc.dram_tensor("wfT", (128, KC, 128), bf16, kind="ExternalInput")
    outp = nc.dram_tensor("outp", (b, KC, 128, s), bf16, kind="ExternalOutput")

    with tile.TileContext(nc) as tc, ExitStack() as ctx:
        const = ctx.enter_context(tc.tile_pool(name="const", bufs=1))
        xpool = ctx.enter_context(tc.tile_pool(name="xp", bufs=2))
        proj = ctx.enter_context(tc.tile_pool(name="proj", bufs=2))
        work = ctx.enter_context(tc.tile_pool(name="work", bufs=3))
        osbp = ctx.enter_context(tc.tile_pool(name="osb", bufs=6))
        opool = ctx.enter_context(tc.tile_pool(name="op", bufs=3))
        psum = ctx.enter_context(
            tc.tile_pool(name="ps", bufs=1, space=bass.MemorySpace.PSUM)
        )

        wq_sb = const.tile([128, KC, 128], bf16, tag="wq")
        wk_sb = const.tile([128, KC, 128], bf16, tag="wk")
        wv_sb = const.tile([128, KC, 128], bf16, tag="wv")
        wf_sb = const.tile([128, KC, 128], bf16, tag="wf")
        nc.sync.dma_start(out=wq_sb, in_=wqT[:].rearrange("k p m -> p k m"))
        nc.sync.dma_start(out=wk_sb, in_=wkT[:].rearrange("k p m -> p k m"))
        nc.sync.dma_start(out=wv_sb, in_=wvT[:].rearrange("k p m -> p k m"))
        nc.sync.dma_start(out=wf_sb, in_=wfT[:])

        st = [dict() for _ in range(b)]

        NXH = 2 if s >= 2 * NQ else 1   # x s-split factor
        HS = s // NXH

        # distinct DGE queues per (batch, s-half) so descriptor setup for the
        # 32 input-chunk DMAs runs in parallel instead of serializing on one
        # sequencer (~0.6us each)
        _XQ = {(0, 0): "sync", (0, 1): "scalar", (1, 0): "gpsimd", (1, 1): "sync"}

        def load_x(ib):
            # split by s-half so the first QKV chunks start after ~1/2 the load
            chunks = {}
            hs = HS
            for sh in range(NXH):
                eng = getattr(nc, _XQ.get((ib % 2, sh % 2), "sync"))
                for kc in range(KC):
                    xc = xpool.tile(
                        [128, hs], bf16, tag=f"x{kc}_{sh}", name="x_chunk"
                    )
                    eng.dma_start(out=xc, in_=xT[ib, kc, :, sh * hs : (sh + 1) * hs])
                    chunks[(kc, sh)] = xc
            st[ib]["x"] = chunks

        def alloc_qkv(ib):
            st[ib]["qT"] = proj.tile([128, s], bf16, tag="qT", name="qT")
            st[ib]["kT"] = proj.tile([128, s], bf16, tag="kT", name="kT")
            st[ib]["v"] = proj.tile([128, SB, HPC * VW], bf16, tag="v", name="v_sb")

        # ---- QKV projection units ------------------------------------------
        def qk_chunk(ib, which, sc):
            w_sb = wq_sb if which == "q" else wk_sb

            def emit():
                x_sb = st[ib]["x"]
                dst = st[ib][which + "T"]
                sh, off = divmod(sc * NQ, HS)
                ps = psum.tile([128, NQ], f32, tag="mm", bufs=2, name="mm_ps")
                for kc in range(KC):
                    nc.tensor.matmul(
                        ps, w_sb[:, kc, :], x_sb[(kc, sh)][:, off : off + NQ],
                        start=(kc == 0), stop=(kc == KC - 1),
                    )
                nc.vector.tensor_copy(out=dst[:, sc * NQ : (sc + 1) * NQ], in_=ps)
            return emit

        def v_units(ib):
            units = []

            def ones_cols():
                v_sb = st[ib]["v"]
                nc.vector.memset(v_sb[:, :, DH : DH + 1], 1.0)
                nc.vector.memset(v_sb[:, :, DH + VW : DH + VW + 1], 1.0)

            units.append(ones_cols)

            def v_block(sbi):
                def emit():
                    x_sb = st[ib]["x"]
                    v_sb = st[ib]["v"]
                    sh, off = divmod(sbi * 128, HS)
                    ps = psum.tile([128, 128], f32, tag="mm", bufs=2, name="mm_ps")
                    for kc in range(KC):
                        nc.tensor.matmul(
                            ps, x_sb[(kc, sh)][:, off : off + 128], wv_sb[:, kc, :],
                            start=(kc == 0), stop=(kc == KC - 1),
                        )
                    nc.vector.tensor_copy(
                        out=v_sb[:, sbi, 0:DH], in_=ps[:, 0:DH]
                    )
                    nc.vector.tensor_copy(
                        out=v_sb[:, sbi, VW : VW + DH], in_=ps[:, DH : 2 * DH]
                    )
                return emit

            for sbi in range(SB):
                units.append(v_block(sbi))
            return units

        # ---- attention group (h, qc): 8 pair-steps + finalize --------------
        def group_units(ib, h, qc):
            hsl = slice(h * DH, (h + 1) * DH)
            qsl = slice(qc * NQ, (qc + 1) * NQ)
            g = {}
            units = []

            def step(kp):
                # 2 score matmuls + one [128, 2*NQ] exp; attnV of the
                # previous pair is emitted after the scores so the PE
                # never waits on the current exp
                def emit():
                    qT, kT, v_sb = st[ib]["qT"], st[ib]["kT"], st[ib]["v"]
                    if kp == 0:
                        g["o"] = psum.tile(
                            [DH + 1, NQ], f32, tag="o", bufs=2, name="o_ps"
                        )
                        g["prev"] = None
                    s2 = psum.tile([128, 2 * NQ], f32, tag="s", bufs=2, name="s2_ps")
                    for half in range(2):
                        kb = 2 * kp + half
                        nc.tensor.matmul(
                            s2[:, half * NQ : (half + 1) * NQ],
                            kT[hsl, kb * 128 : (kb + 1) * 128],
                            qT[hsl, qsl],
                            start=True, stop=True,
                        )
                    p2 = work.tile([128, 2 * NQ], bf16, tag="p", bufs=4, name="p2")
                    nc.scalar.activation(out=p2, in_=s2, func=Exp)
                    if g["prev"] is not None:
                        pkp, pp = g["prev"]
                        for half in range(2):
                            kb = 2 * pkp + half
                            nc.tensor.matmul(
                                g["o"], v_sb[:, kb, h * VW : (h + 1) * VW],
                                pp[:, half * NQ : (half + 1) * NQ],
                                start=(kb == 0), stop=False,
                            )
                    g["prev"] = (kp, p2)
                return emit

            def fin():
                def emit():
                    v_sb = st[ib]["v"]
                    pkp, pp = g["prev"]
                    for half in range(2):
                        kb = 2 * pkp + half
                        nc.tensor.matmul(
                            g["o"], v_sb[:, kb, h * VW : (h + 1) * VW],
                            pp[:, half * NQ : (half + 1) * NQ],
                            start=False, stop=(half == 1),
                        )
                    o_sb = osbp.tile([DH + 1, NQ], f32, tag="osb", name="o_sb")
                    nc.vector.tensor_copy(out=o_sb, in_=g["o"])
                    st[ib][("o", h, qc)] = o_sb
                return emit

            for kp in range(SB // 2):
                units.append(step(kp))
            units.append(fin())
            return units

        # ---- per-qc normalization + final projection -----------------------
        def norm_qc(ib, qc):
            def emit():
                qsl = slice(qc * NQ, (qc + 1) * NQ)
                ffr = st[ib]["ffr"]
                for h in range(HPC):
                    o_sb = st[ib][("o", h, qc)]
                    dnrow = work.tile([1, NQ], f32, tag="dnrow", bufs=3, name="dnrow")
                    nc.gpsimd.dma_start(out=dnrow, in_=o_sb[DH : DH + 1, :])
                    rr = work.tile([1, NQ], f32, tag="rr", bufs=3, name="rr")
                    nc.vector.reciprocal_approx_fast(out=rr, in_=dnrow)
                    rdbc = work.tile([DH, NQ], f32, tag="rdbc", bufs=3, name="rdbc")
                    nc.gpsimd.partition_broadcast(rdbc, rr)
                    nc.vector.tensor_mul(
                        out=ffr[h * DH : (h + 1) * DH, qsl],
                        in0=o_sb[0:DH, :],
                        in1=rdbc,
                    )
            return emit

        def ff_qc(ib, qc):
            units = []

            def one(j):
                def emit():
                    qsl = slice(qc * NQ, (qc + 1) * NQ)
                    ps = psum.tile([128, NQ], f32, tag="mm", bufs=2, name="mm_ps")
                    nc.tensor.matmul(
                        ps, wf_sb[:, j, :], st[ib]["ffr"][:, qsl],
                        start=True, stop=True,
                    )
                    f_sb = opool.tile([128, NQ], bf16, tag="f", name="f_sb")
                    nc.vector.tensor_copy(out=f_sb, in_=ps)
                    nc.sync.dma_start(out=outp[ib, j, :, qsl], in_=f_sb)
                return emit

            for j in range(KC):
                units.append(one(j))
            return units

        # ---- attention stream: qc-major with streamed norm/ff --------------
        def attn_stream(ib, deferred_qT=False):
            st[ib]["ffr"] = proj.tile([128, s], bf16, tag="ffr", name="ffr")
            qc_blocks = []
            for qc in range(SQ):
                g0 = group_units(ib, 0, qc)
                g1 = group_units(ib, 1, qc)
                # h1's first score-pair ahead of h0's finalize keeps the exp
                # stream dense across the group boundary
                attn = g0[:-1] + [g1[0], g0[-1]] + g1[1:]
                if deferred_qT and qc + 1 < SQ:
                    attn.append(qk_chunk(ib, "q", qc + 1))
                tail = [norm_qc(ib, qc)] + ff_qc(ib, qc)
                qc_blocks.append((attn, tail))
            # spread each qc's norm/ff tail thinly across the next qc's steps
            units = []
            pend = None
            for attn, tail in qc_blocks:
                if pend:
                    units.extend(_interleave(attn, pend, skip=2))
                else:
                    units.extend(attn)
                pend = tail
            units.extend(pend)
            return units

        # ---- emission schedule ---------------------------------------------
        load_x(0)
        alloc_qkv(0)
        vu0 = v_units(0)
        ones0, vb0 = vu0[0], vu0[1:]
        # minimal prologue: everything group (h0, qc=0) consumes, in need-order
        prologue = [ones0, qk_chunk(0, "k", 0)]
        prologue.extend(vb0[0:4])
        prologue.append(qk_chunk(0, "q", 0))
        for i in range(1, SQ):
            prologue.append(qk_chunk(0, "k", i))
            prologue.extend(vb0[4 * i : 4 * (i + 1)])
        for u in prologue:
            u()

        if b > 1:
            load_x(1)
            alloc_qkv(1)
            vu = v_units(1)
            ones_u, vb = vu[0], vu[1:]
            # remaining b0 q-chunks first (needed from qc=1 on), then batch-1
            # qkv in the order its attention will need it
            fillers = [qk_chunk(0, "q", 1), qk_chunk(0, "q", 2), qk_chunk(0, "q", 3)]
            fillers += [ones_u, qk_chunk(1, "k", 0)]
            fillers.extend(vb[0:4])
            fillers.append(qk_chunk(1, "q", 0))
            for i in range(1, SQ):
                fillers.append(qk_chunk(1, "k", i))
                fillers.extend(vb[4 * i : 4 * (i + 1)])
            s0 = attn_stream(0)
            head0, tail0 = s0[:-9], s0[-9:]
            for u in _interleave(head0, fillers):
                u()
            s1 = attn_stream(1, deferred_qT=True)
            for u in _interleave(s1[:27], tail0):
                u()
            for u in s1[27:]:
                u()
        else:
            for u in attn_stream(0):
                u()

    nc.compile()
    return nc


def make_in_maps(x, Wq, Wk, Wv, Wff, n_cores=NCORES):
    """Per-core input dicts. Core c owns heads (2c, 2c+1) = D dims [128c, 128c+128)."""
    x = np.asarray(x, dtype=np.float32)
    b, s, d = x.shape
    KC = d // 128
    xT = np.ascontiguousarray(x.transpose(0, 2, 1)).reshape(b, KC, 128, s).astype(BF16)
    scale = 1.0 / np.sqrt(d)
    in_maps = []
    for c in range(n_cores):
        sl = slice(128 * c, 128 * (c + 1))
        wq = np.ascontiguousarray((np.asarray(Wq)[sl, :] * scale).T).reshape(KC, 128, 128)
        wk = np.ascontiguousarray(np.asarray(Wk)[sl, :].T).reshape(KC, 128, 128)
        wv = np.ascontiguousarray(np.asarray(Wv)[sl, :].T).reshape(KC, 128, 128)
        wf = np.ascontiguousarray(np.asarray(Wff)[:, sl].T).reshape(128, KC, 128)
        in_maps.append(
            {
                "xT": xT,
                "wqT": wq.astype(BF16),
                "wkT": wk.astype(BF16),
                "wvT": wv.astype(BF16),
                "wfT": wf.astype(BF16),
            }
        )
    return in_maps


def gather(results, bff, b=B, s=S, d=D):
    total = np.zeros((b, d // 128, 128, s), np.float32)
    for r in results:
        total += r["outp"].astype(np.float32)
    out = total.reshape(b, d, s).transpose(0, 2, 1)
    return (out + np.asarray(bff, np.float32)[None, None, :]).astype(np.float32)


_CACHE = {}


def kernel(x, Wq, Wk, Wv, Wff, bff):
    from concourse.bass_utils import run_bass_kernel_spmd

    x = np.asarray(x, np.float32)
    b, s, d = x.shape
    key = (b, s, d)
    if key not in _CACHE:
        _CACHE[key] = build_program(b, s, d)
    nc = _CACHE[key]
    in_maps = make_in_maps(x, Wq, Wk, Wv, Wff)
    res = run_bass_kernel_spmd(nc, in_maps, list(range(NCORES)))
    return gather(res.results, bff, b, s, d)
